# revision 23
# baseline (speedup 1.0000x reference)
"""Multi-head attention (b=4, c=256, l=2048, 8 heads x 64) on 8 TRN2 NeuronCores.

Sharding: core i handles batch b = i//2 and query half qh = i%2 (1024 queries),
computing all 8 heads over the full 2048-key context. Outputs are disjoint
[256, 1024] slabs -> host-side concat only, no collectives.

v2 structure (vs the earlier phase-split kernel):
  - One unified stream of 8 SEGMENTS = single heads, 11 score tiles each
    ([128,1536] psum, chunks ordered qh-outer/jt-inner). exp runs on ScalarE
    with the 1/8 score scale folded into ACT's free affine (scale=).
  - The Scalar queue carries NOTHING but exp (one warm-up table load; no LN
    -> no mid-stream ACT_TABLE_LOADs; no DMAs/copies/muls/adds).
  - QKV projections are [128,512] psum "pieces" that borrow the qk slots
    (explicit qka/qkb parity tags keep score tiles double-buffered no
    matter what is interleaved), scheduled against DMA ETAs; PV flushes
    lag 10 tiles behind the exp stream.
  - All softmax denominators use the 1-instruction DVE reciprocal_approx_fast
    (~51 ULP, 5x cheaper than reciprocal; denominators are benign sums).
  - Last head splits normalization by query half: qh0's chain + out-proj
    + DMA overlap the tail; only qh1's short chain is serial.
  - DMAs ride the sync/gpsimd queues (plus one early x chunk on the
    scalar queue right after the warm-up, while ACT would idle anyway).
"""

import sys

if "/opt/trn_rl_repo" not in sys.path:
    sys.path.insert(0, "/opt/trn_rl_repo")

import numpy as np

import concourse.bass as bass
import concourse.mybir as mybir
import concourse.tile as tile
from concourse import bacc
from concourse.bass_utils import run_bass_kernel_spmd

F32 = mybir.dt.float32
BF16 = mybir.dt.bfloat16
EXP = mybir.ActivationFunctionType.Exp
MULT = mybir.AluOpType.mult

B, C, L = 4, 256, 2048
H, D = 8, 64
HID = H * D  # 512
LQ = L // 2  # 1024 queries per core
NJT = L // 128  # 16 key tiles
SCALE = D**-0.5
LAG = 10  # PV flush lag in score tiles

_cached = {}

# Fitted on [-0.32, 0.32] (relative-weighted LSQ): e^y ~ 1 + y(b1 + y(b2 + y*b3))
# with y = SCALE*s/32; then e^{SCALE*s} = (1+t)^32 via 5 squarings.
_KY = SCALE / 32.0
_B1 = 1.0001788898220774 * _KY
_B2 = 0.5030173946968963 * _KY * _KY
_B3 = 0.16434063923831171 * _KY * _KY * _KY


def _register_dve_exp():
    """Register 2-instruction DVE exp: poly then (1+t)^32. Uses the
    documented extension point (dve_ops.OPS) at runtime, shas computed
    in-process so codegen and table-gen agree."""
    import concourse.dve_ops as dvo
    from concourse.dve_spec import Spec, Src0, C0, C1, C2, One, sq
    from concourse.dve_spec import lower as dve_lower
    from concourse.dve_uop import DveOpSpec

    if "EXP_POLY_ANT" in dvo._SUB_OPCODE_FOR_NAME:
        by = {o.name: o for o in dvo.OPS}
        return by["EXP_POLY_ANT"], by["EXP_SQ32_ANT"]

    def ref_poly(in0, in1, s0, s1, imm2):
        return (((in0 * s0 + s1) * in0 + imm2) * in0).astype(np.float32)

    def ref_sq32(in0, in1, s0, s1, imm2):
        y = (in0 + np.float32(1.0)).astype(np.float32)
        for _ in range(5):
            y = (y * y).astype(np.float32)
        return y

    out = []
    for name, body, ref in [
        ("EXP_POLY_ANT", ((Src0 * C0 + C1) * Src0 + C2) * Src0, ref_poly),
        ("EXP_SQ32_ANT", sq(sq(sq(sq(sq(Src0 + One))))), ref_sq32),
    ]:
        opcode = dvo._CUSTOM_DVE_ROW_BASE + len(dvo.OPS)
        spec = Spec(body=body, reference=ref)
        shas = {}
        for ver in ("v3", "v4"):
            u = dve_lower(spec, ver=ver)
            shas[ver] = DveOpSpec(name=name, opcode=opcode, uops=u, rd1_en=False).sha(ver)
        op = dvo.DveOp(name, spec, subdim=False, uops_sha=shas)
        dvo.OPS.append(op)
        dvo._SUB_OPCODE_FOR_NAME[name] = opcode
        dvo.CUSTOM_DVE_SPECS[name] = spec
        out.append(op)
    return out[0], out[1]


# score tiles whose exp runs on DVE instead of ACT (mid-stream, away from
# head boundaries so pons copies / piece deadlines are unaffected)
DVE_TILES = frozenset({20, 26, 31, 35, 39, 47, 51, 59, 63, 71, 75})


def build_nc():
    nc = bacc.Bacc(
        "TRN2",
        target_bir_lowering=False,
        debug=False,
        enable_asserts=False,
        num_devices=8,
    )
    x_d = nc.dram_tensor("x", [C, L], BF16, kind="ExternalInput")
    xq_d = nc.dram_tensor("xq", [C, LQ], BF16, kind="ExternalInput")
    wq_d = nc.dram_tensor("wqkvT", [C, 3 * HID], BF16, kind="ExternalInput")
    wo_d = nc.dram_tensor("woutT", [HID, C], BF16, kind="ExternalInput")
    bias_d = nc.dram_tensor("bias", [C, 1], F32, kind="ExternalInput")
    out_d = nc.dram_tensor("out", [C, LQ], BF16, kind="ExternalOutput")

    EXP_POLY, EXP_SQ32 = _register_dve_exp()

    with tile.TileContext(nc) as tc:
        with (
            tc.tile_pool(name="const", bufs=1) as cp,
            tc.tile_pool(name="epool", bufs=13) as ep,
            tc.tile_pool(name="rpool", bufs=2) as rp,
            tc.tile_pool(name="opool", bufs=2) as op,
            tc.tile_pool(name="qkps", bufs=1, space=bass.MemorySpace.PSUM) as qkps,
            tc.tile_pool(name="pvps", bufs=1, space=bass.MemorySpace.PSUM) as pvps,
        ):
            # ---- persistent SBUF tensors ----
            xb = [cp.tile([128, L], BF16, tag=f"xb{k}", name=f"xb{k}") for k in range(2)]
            xq = [cp.tile([128, LQ], BF16, tag=f"xq{k}", name=f"xq{k}") for k in range(2)]
            wq = [cp.tile([128, 3 * HID], BF16, tag=f"wq{k}", name=f"wq{k}") for k in range(2)]
            wo = [cp.tile([128, C], BF16, tag=f"wo{k}", name=f"wo{k}") for k in range(4)]
            bias = [cp.tile([128, 1], F32, tag=f"bias{k}", name=f"bias{k}") for k in range(2)]
            Qs = [cp.tile([128, LQ], BF16, tag=f"Q{m}", name=f"Q{m}") for m in range(4)]
            Ks = [cp.tile([128, L], BF16, tag=f"K{m}", name=f"K{m}") for m in range(4)]
            VT = [cp.tile([128, H, D + 1], BF16, tag=f"VT{t}", name=f"VT{t}") for t in range(NJT)]
            attn = [cp.tile([128, LQ], BF16, tag=f"attn{m}", name=f"attn{m}") for m in range(4)]
            dum = cp.tile([1, 16], F32, tag="dum", name="dum")
            dumo = cp.tile([1, 16], BF16, tag="dumo", name="dumo")

            # ACT table warm-up: first (and only non-exp-stream) Scalar work.
            nc.gpsimd.memset(dum[:], 1.0)
            nc.scalar.activation(dumo[:], dum[:], EXP)

            # VT ones columns (feed the PV denominator row) - off critical path
            for t in range(NJT):
                nc.gpsimd.memset(VT[t][:, :, D : D + 1], 1.0)

            # ---- DMA issue order (ETA-priority) ----
            # Only sync/gpsimd/scalar can issue DMAs. sync+gpsimd carry the
            # critical path; scalar takes just x-c2 right after the table
            # load (ACT is idle then anyway), giving a 3rd early queue.
            qs = [nc.sync, nc.gpsimd]  # per row-half for [C, *] tensors
            rows = [slice(0, 128), slice(128, 256)]
            for k in range(2):
                qs[k].dma_start(wq[k][:, HID : HID + 128], wq_d.ap()[rows[k], HID : HID + 128])
            for k in range(2):
                qs[k].dma_start(wq[k][:, 0:128], wq_d.ap()[rows[k], 0:128])
            for k in range(2):
                qs[k].dma_start(xq[k][:, 0:512], xq_d.ap()[rows[k], 0:512])
            for c in (0, 3):
                for k in range(2):
                    qs[k].dma_start(
                        xb[k][:, 512 * c : 512 * (c + 1)],
                        x_d.ap()[rows[k], 512 * c : 512 * (c + 1)],
                    )
            for c in (1, 2):  # x c1/c2 on the scalar queue (post-warmup)
                for k in range(2):
                    nc.scalar.dma_start(
                        xb[k][:, 512 * c : 512 * (c + 1)],
                        x_d.ap()[rows[k], 512 * c : 512 * (c + 1)],
                    )
            for k in range(2):
                qs[k].dma_start(xq[k][:, 512:1024], xq_d.ap()[rows[k], 512:1024])
            for k in range(2):
                qs[k].dma_start(wq[k][:, 2 * HID : 3 * HID], wq_d.ap()[rows[k], 2 * HID : 3 * HID])
            for k in range(2):
                qs[k].dma_start(wq[k][:, 128:HID], wq_d.ap()[rows[k], 128:HID])
            for k in range(2):
                qs[k].dma_start(wq[k][:, HID + 128 : 2 * HID], wq_d.ap()[rows[k], HID + 128 : 2 * HID])
            for k in range(4):
                qs[k % 2].dma_start(wo[k][:], wo_d.ap()[128 * k : 128 * (k + 1), :])
            for k in range(2):
                qs[k].dma_start(bias[k][:], bias_d.ap()[rows[k], :])

            # ---- side-work pieces (borrow one qk psum slot each) ----
            # Score tiles use explicit alternating tags (qka/qkb) keyed by
            # tile parity, so double-buffering survives any piece insertion;
            # pieces alternate tags independently.
            pc = [0]

            def ptag():
                pc[0] += 1
                return "qka" if pc[0] % 2 else "qkb"

            def kq_piece(m, col, src, dst, wcol):
                ps = qkps.tile([128, 512], F32, tag=ptag(), name="pp")
                for k in range(2):
                    nc.tensor.matmul(
                        ps[:],
                        wq[k][:, wcol + 128 * m : wcol + 128 * (m + 1)],
                        src[k][:, 512 * col : 512 * (col + 1)],
                        start=(k == 0),
                        stop=(k == 1),
                    )
                nc.vector.tensor_copy(dst[m][:, 512 * col : 512 * (col + 1)], ps[:])

            def vt_piece(t):
                ps = qkps.tile([128, 512], F32, tag=ptag(), name="pv")
                for k in range(2):
                    nc.tensor.matmul(
                        ps[:],
                        xb[k][:, 128 * t : 128 * (t + 1)],
                        wq[k][:, 2 * HID : 3 * HID],
                        start=(k == 0),
                        stop=(k == 1),
                    )
                nc.vector.tensor_copy(
                    VT[t][:, :, 0:D], ps[:].rearrange("p (h c) -> p h c", h=H)
                )

            def kq_piece2(m, cp2, src, dst, wcol):
                ps = qkps.tile([128, 1024], F32, tag=ptag(), name="pp2")
                for k in range(2):
                    nc.tensor.matmul(
                        ps[:],
                        wq[k][:, wcol + 128 * m : wcol + 128 * (m + 1)],
                        src[k][:, 1024 * cp2 : 1024 * (cp2 + 1)],
                        start=(k == 0),
                        stop=(k == 1),
                    )
                nc.vector.tensor_copy(dst[m][:, 1024 * cp2 : 1024 * (cp2 + 1)], ps[:])

            def KP(m, col):
                return lambda: kq_piece(m, col, xb, Ks, HID)

            def KP2(m, cp2):
                return lambda: kq_piece2(m, cp2, xb, Ks, HID)

            def QP(m, col):
                return lambda: kq_piece(m, col, xq, Qs, 0)

            def QP2(m):
                return lambda: kq_piece2(m, 0, xq, Qs, 0)

            def VP(t):
                return lambda: vt_piece(t)

            # pieces by GLOBAL score-tile index; pieces run AFTER that
            # tile's QK, so a piece feeding tile T's QK must sit at <= T-1.
            side_at = {
                0: [KP(0, 1)],  # K c1 needed by tile-1 QK (jt4)
                1: [KP(0, 2)],  # K c2 needed by tile-2 QK (jt8); x-c2 ETA ~13
                3: [KP(0, 3)],  # K c3 needed by tile-4 QK (jt12)
                4: [QP(0, 1)],  # Q qh1 needed by tile-5 QK (chunk 16)
                6: [VP(0), VP(1)],
                7: [VP(2), VP(3)],
                8: [VP(4), VP(5)],
                9: [VP(6), VP(7)],
                10: [VP(8), VP(9)],
                11: [VP(10), VP(11)],
                12: [VP(12), VP(13)],
                13: [VP(14), VP(15)],
                14: [KP(1, 0)],
                15: [KP(1, 1)],
                16: [KP(1, 2)],
                17: [KP(1, 3)],
                18: [QP(1, 0)],
                19: [QP(1, 1)],
                24: [KP(2, 0)],
                25: [KP(2, 1)],
                26: [KP(2, 2)],
                27: [KP(2, 3)],
                28: [QP(2, 0)],
                29: [QP(2, 1)],
                36: [KP(3, 0), KP(3, 1)],
                38: [KP(3, 2), KP(3, 3)],
                40: [QP(3, 0), QP(3, 1)],
            }

            # ---- normalization (DVE + GpSimd only; approx reciprocal) ----
            def norm_chain(h, po, cols, width):
                p, s = h // 2, h % 2
                pons = rp.tile([D, width], F32, tag="pon", name="pons")
                nc.vector.tensor_copy(pons[:], po[0:D, cols])
                den0 = rp.tile([1, width], F32, tag="den", name="den0")
                nc.vector.tensor_copy(den0[:], po[D : D + 1, cols])
                rec = rp.tile([1, width], F32, tag="rec", name="rec")
                nc.vector.reciprocal_approx_fast(rec[:], den0[:])
                rbc = rp.tile([64, width], F32, tag="rbc", name="rbc")
                nc.gpsimd.partition_broadcast(rbc[:], rec[:])

                def mult():
                    nc.vector.tensor_tensor(
                        attn[p][64 * s : 64 * (s + 1), cols],
                        pons[:],
                        rbc[:],
                        MULT,
                    )

                return mult

            # ---- main stream ----
            def chunk_qk(ps, ci, h, c):
                p, s = h // 2, h % 2
                qh, jt = divmod(c, 16)
                nc.tensor.matmul(
                    ps[:, 512 * ci : 512 * (ci + 1)],
                    Ks[p][64 * s : 64 * (s + 1), 128 * jt : 128 * (jt + 1)],
                    Qs[p][64 * s : 64 * (s + 1), 512 * qh : 512 * (qh + 1)],
                    start=True,
                    stop=True,
                )

            def pv_flush(E, h, c0, nch, po):
                for ci in range(nch):
                    qh, jt = divmod(c0 + ci, 16)
                    nc.tensor.matmul(
                        po[:, 512 * qh : 512 * (qh + 1)],
                        VT[jt][:, h, :],
                        E[:, 512 * ci : 512 * (ci + 1)],
                        start=(jt == 0),
                        stop=(jt == NJT - 1),
                    )

            pending = []  # FIFO of (E, h, c0, nch, po)
            deferred = []  # deferred norm mults (run next tile)
            h7_qh0 = []

            def flush_one():
                if not pending:
                    return
                E, h, c0, nch, po = pending.pop(0)
                pv_flush(E, h, c0, nch, po)
                if h < 7 and c0 + nch == 32:
                    deferred.append(norm_chain(h, po, slice(0, LQ), LQ))
                elif h == 7 and c0 <= 15 < c0 + nch:
                    h7_qh0.append(norm_chain(7, po, slice(0, 512), 512))

            # prologue pieces: Q cols 0:512 then K cols 0:512 (earliest
            # DMA arrivals first; K01 rides side_at[0] for tile-1's QK).
            kq_piece(0, 0, xq, Qs, 0)
            kq_piece(0, 0, xb, Ks, HID)

            gt = 0  # global score-tile index
            po7 = None
            for h in range(8):
                po = pvps.tile([D + 1, LQ], F32, tag="pv", name="po")
                if h == 7:
                    po7 = po
                c0 = 0
                while c0 < 32:
                    nch = min(3, 32 - c0)
                    ps = qkps.tile([128, 512 * nch], F32, tag="qka" if gt % 2 == 0 else "qkb", name="psqk")
                    for ci in range(nch):
                        chunk_qk(ps, ci, h, c0 + ci)
                    if gt >= LAG:
                        flush_one()
                        if gt >= 36 and gt % 4 == 1 and len(pending) > 1:
                            flush_one()
                        if gt >= 70 and gt % 2 == 1 and len(pending) > 1:
                            flush_one()
                        if gt >= 78 and len(pending) > 1:
                            flush_one()
                    if deferred:
                        deferred.pop(0)()
                    for piece in side_at.pop(gt, ()):
                        piece()
                    E = ep.tile([128, 512 * nch], BF16, tag="e", name="E")
                    if gt in DVE_TILES:
                        esc = rp.tile([128, 512 * nch], F32, tag="esc", name="esc")
                        nc.vector._custom_dve(
                            EXP_POLY, out=esc[:], in0=ps[:], s0=_B3, s1=_B2, imm2=_B1
                        )
                        nc.vector._custom_dve(EXP_SQ32, out=E[:], in0=esc[:])
                    else:
                        nc.scalar.activation(E[:], ps[:], EXP, scale=SCALE)
                    pending.append((E, h, c0, nch, po))
                    c0 += nch
                    gt += 1
            assert not side_at, side_at
            while pending:
                flush_one()
            for m_ in deferred:
                m_()
            # qh1 chain emitted FIRST so its recip/bcast aren't queued
            # behind qh0's mult and the out-proj biases on the DVE queue.
            mult1 = norm_chain(7, po7, slice(512, 1024), 512)
            h7_qh0[0]()

            def out_proj0():
                for m in range(2):
                    ps = qkps.tile([128, 512], F32, tag=ptag(), name="pso")
                    for k in range(4):
                        nc.tensor.matmul(
                            ps[:],
                            wo[k][:, 128 * m : 128 * (m + 1)],
                            attn[k][:, 0:512],
                            start=(k == 0),
                            stop=(k == 3),
                        )
                    osb = op.tile([128, 512], BF16, tag="osb", name="osb")
                    nc.vector.tensor_scalar_add(osb[:], ps[:], bias[m][:])
                    eng = [nc.sync, nc.gpsimd][m]
                    eng.dma_start(out_d.ap()[128 * m : 128 * (m + 1), 0:512], osb[:])

            # out-proj for cols 512:1024: heads 0-6 contraction runs BEFORE
            # the final norm lands; only head-7's 64 hidden rows come after.
            # Both m-chains live in one [128,1024] tile in the pv slot
            # (free at tail), as two independent 512-col regions.
            ps1t = pvps.tile([128, LQ], F32, tag="pv", name="ps1t")
            ps1 = [ps1t[:, 0:512], ps1t[:, 512:1024]]
            for m in range(2):
                for k in range(3):
                    nc.tensor.matmul(
                        ps1[m],
                        wo[k][:, 128 * m : 128 * (m + 1)],
                        attn[k][:, 512:1024],
                        start=(k == 0),
                        stop=False,
                    )
                nc.tensor.matmul(
                    ps1[m],
                    wo[3][0:64, 128 * m : 128 * (m + 1)],
                    attn[3][0:64, 512:1024],
                    start=False,
                    stop=False,
                )

            out_proj0()
            mult1()
            for m in range(2):
                nc.tensor.matmul(
                    ps1[m],
                    wo[3][64:128, 128 * m : 128 * (m + 1)],
                    attn[3][64:128, 512:1024],
                    start=False,
                    stop=True,
                )
                osb = op.tile([128, 512], BF16, tag="osb", name="osb")
                nc.vector.tensor_scalar_add(osb[:], ps1[m], bias[m][:])
                for c2 in range(2):
                    eng = [nc.sync, nc.gpsimd][(m + c2) % 2]
                    eng.dma_start(
                        out_d.ap()[128 * m : 128 * (m + 1), 512 + 256 * c2 : 512 + 256 * (c2 + 1)],
                        osb[:, 256 * c2 : 256 * (c2 + 1)],
                    )

    nc.compile()
    return nc


def get_nc():
    if "nc" not in _cached:
        _cached["nc"] = build_nc()
    return _cached["nc"]


def make_in_maps(x, w_qkv, w_out, b_out):
    import ml_dtypes

    bf16 = ml_dtypes.bfloat16
    wqkvT = np.ascontiguousarray(w_qkv.T.astype(bf16))
    woutT = np.ascontiguousarray(w_out.T.astype(bf16))
    bias = np.ascontiguousarray(b_out.astype(np.float32).reshape(C, 1))
    in_maps = []
    for i in range(8):
        b, qh = i // 2, i % 2
        xb = np.ascontiguousarray(x[b].astype(bf16))
        xq = np.ascontiguousarray(xb[:, qh * LQ : (qh + 1) * LQ])
        in_maps.append(
            {"x": xb, "xq": xq, "wqkvT": wqkvT, "woutT": woutT, "bias": bias}
        )
    return in_maps


def assemble(results):
    out = np.empty((B, C, L), dtype=np.float32)
    for i in range(8):
        b, qh = i // 2, i % 2
        out[b][:, qh * LQ : (qh + 1) * LQ] = np.asarray(
            results[i]["out"], dtype=np.float32
        )
    return out


def kernel(x, w_qkv, w_out, b_out):
    x = np.asarray(x, dtype=np.float32)
    w_qkv = np.asarray(w_qkv, dtype=np.float32)
    w_out = np.asarray(w_out, dtype=np.float32)
    b_out = np.asarray(b_out, dtype=np.float32)
    assert x.shape == (B, C, L), x.shape
    nc = get_nc()
    in_maps = make_in_maps(x, w_qkv, w_out, b_out)
    res = run_bass_kernel_spmd(nc, in_maps, list(range(8)), trace=False)
    return assemble(res.results)
